# revision 1
# baseline (speedup 1.0000x reference)
"""Trainium2 Bass kernel for nn_AttentionBlock (B=2, S=4096, HID=256, 8 heads).

Sharding: 8 cores = 2 batches x 4 query-chunks of 1024 queries.
Each core redundantly computes full K/V projections for its batch (over the
mask-compacted key set), then attention for its 1024 queries over all 8
heads, then the output projection. Host gathers by concatenation.

Key structure (v2):
- Mask compaction: surviving key indices gathered via indirect DMA from a
  host-concatenated [K|V] tensor with appended zero rows; padding slots
  point at the zero rows, and the ones-column of the augmented V carries the
  mask, so masking is exact with no exp bias anywhere.
- Scores computed transposed (sT[k,q]) via 4-way row-packed K=32 fp16
  matmuls, 512 queries per matmul.
- Softmax exp split across engines per head-pair: half the head-pairs use
  the Scalar engine's LUT exp; the other half use a two-sample Schraudolph
  bit-trick exp on the Vector engine (t = A*x + B rounded to int16,
  bitcast to fp16 = 2^(t/1024) approx; summing the B and B+512 samples
  cancels the sawtooth to ~0.5% ripple; the per-softmax constant factor
  cancels in normalization). The extra sample is accumulated by the PE via
  a second PV matmul wave into the same PSUM accumulators.
- Augmented V tiles [128, 512]: per head 64 cols = [32 v | mask | 31 zeros],
  so M=64 PV matmuls fully cover the PSUM rows (no garbage rows) and the
  denominators accumulate alongside.
- Normalization: denominator rows DMA-packed into one [8,512] tile per qc,
  reciprocal via int32 magic-constant + 1 Newton iteration on DVE (scaled
  by 2048 to keep fp16 r values normal), scattered to partitions 32/33 of a
  small tile, broadcast to all 128 partitions with one K=2 matmul against a
  1/2048-valued selector, then one full-tile multiply produces the fp16
  normalized weights for the output projection.
- Output projection: fused K=128 matmuls against zero-padded Wo rows, bias
  (incl. folded bv@Wo) via a K=1 ones matmul.
"""

import numpy as np

import concourse.bacc as bacc
import concourse.bass as bass
from concourse import mybir
from concourse.tile import TileContext
from concourse.masks import make_identity
from concourse.bass_utils import run_bass_kernel_spmd

F32 = mybir.dt.float32
F16 = mybir.dt.float16
I16 = mybir.dt.int16
I32 = mybir.dt.int32
AF = mybir.ActivationFunctionType
ALU = mybir.AluOpType

HID = 256
HEADS = 8
DH = 32
SK = 4096
SKP = 4104  # K/V rows incl. zero-pad rows
SQ = 1024   # queries per core
SCALE = 1.0 / np.sqrt(32.0)
A16 = 1024.0 / np.log(2.0)          # Schraudolph slope (fp16 format)
ASC = A16 * SCALE                   # folded score scale
B16 = 15360.0                       # Schraudolph offset (15*1024)
MAGIC = 0x7EF311C3                  # fp32 reciprocal magic
RS = 2048.0                         # reciprocal output scaling (keeps fp16 normal)

_CACHE = {}

# exp-engine map: (qc, g, jj) -> True if DVE (Schraudolph), False if ACT
def _use_dve(qc, g, jj):
    return jj == 1


def _build_nc(nkc):
    """nkc = number of 128-key tiles after mask compaction (multiple of 4)."""
    skc = nkc * 128
    nc = bacc.Bacc("TRN2", target_bir_lowering=False, debug=False, num_devices=8)

    q_d = nc.dram_tensor("q_in", [SQ, HID], F16, kind="ExternalInput").ap()
    kvh_d = nc.dram_tensor("kv_in", [SKP, 2 * HID], F16, kind="ExternalInput").ap()
    vg_d = nc.dram_tensor("vginit", [128, nkc * 512], F16, kind="ExternalInput").ap()
    i_d = nc.dram_tensor("idx_in", [128, nkc], I32, kind="ExternalInput").ap()
    wq_d = nc.dram_tensor("wq", [HID, HID], F32, kind="ExternalInput").ap()
    wk_d = nc.dram_tensor("wk", [HID, HID], F32, kind="ExternalInput").ap()
    wv_d = nc.dram_tensor("wv", [HID, HID], F32, kind="ExternalInput").ap()
    wo_d = nc.dram_tensor("wo_arr", [128, 1024], F32, kind="ExternalInput").ap()
    bq_d = nc.dram_tensor("bq2", [128, 2], F32, kind="ExternalInput").ap()
    bk_d = nc.dram_tensor("bk2", [128, 2], F32, kind="ExternalInput").ap()
    bo_d = nc.dram_tensor("bo2", [1, HID], F32, kind="ExternalInput").ap()
    sel_d = nc.dram_tensor("sel2", [2, 128], F16, kind="ExternalInput").ap()
    out_d = nc.dram_tensor("out", [SQ, HID], F32, kind="ExternalOutput").ap()

    from contextlib import ExitStack

    with TileContext(nc) as tc, ExitStack() as top:
        const = top.enter_context(tc.tile_pool(name="const", bufs=1))
        persist = top.enter_context(tc.tile_pool(name="persist", bufs=1))
        io_pool = top.enter_context(tc.tile_pool(name="io", bufs=16))
        xt_pool = top.enter_context(tc.tile_pool(name="xt", bufs=3))
        vt_pool = top.enter_context(tc.tile_pool(name="vt", bufs=3))
        pt_pool = top.enter_context(tc.tile_pool(name="pt", bufs=6))
        wc_pool = top.enter_context(tc.tile_pool(name="wc", bufs=8))
        dn_pool = top.enter_context(tc.tile_pool(name="dn", bufs=18))
        rs_pool = top.enter_context(tc.tile_pool(name="rs", bufs=8))
        osb_pool = top.enter_context(tc.tile_pool(name="osb", bufs=4))

        tpsum = top.enter_context(tc.tile_pool(name="tpsum", bufs=1, space="PSUM"))
        ppsum = top.enter_context(tc.tile_pool(name="ppsum", bufs=1, space="PSUM"))
        st_pool = top.enter_context(tc.tile_pool(name="stp", bufs=2, space="PSUM"))
        wt_pool = top.enter_context(tc.tile_pool(name="wtp", bufs=2, space="PSUM"))

        # ---------------- early IO: idx, gathers, q loads ----------------
        idx_sb = const.tile([128, nkc], I32, name="idx_sb")
        nc.sync.dma_start(idx_sb, i_d)

        xkv_tiles = {}

        def gather_kv(s):
            xkv = io_pool.tile([128, 512], F16, tag="xkv", name="xkv")
            nc.gpsimd.indirect_dma_start(
                out=xkv, out_offset=None, in_=kvh_d,
                in_offset=bass.IndirectOffsetOnAxis(
                    ap=idx_sb[:, s:s + 1], axis=0))
            xkv_tiles[s] = xkv

        for s in range(min(8, nkc)):
            gather_kv(s)

        xq_all = []
        for s in range(8):
            xq = io_pool.tile([128, 256], F16, tag="xq", name="xq")
            nc.sync.dma_start(xq, q_d[s * 128:(s + 1) * 128, :])
            xq_all.append(xq)

        # ---------------- constants ----------------
        wq_hf = []
        wk_hf = []
        wv_hf = []
        for t in range(2):
            for nm, d_ap, lst in (("wq", wq_d, wq_hf), ("wk", wk_d, wk_hf),
                                  ("wv", wv_d, wv_hf)):
                wf = const.tile([128, 256], F32, name=f"{nm}_f{t}")
                nc.sync.dma_start(wf, d_ap[t * 128:(t + 1) * 128, :])
                wb = const.tile([128, 256], F16, name=f"{nm}_h{t}")
                nc.vector.tensor_copy(wb, wf)
                lst.append(wb)
        wo_f = const.tile([128, 1024], F32, name="wo_f")
        nc.scalar.dma_start(wo_f, wo_d)
        wo_hf = const.tile([128, 1024], F16, name="wo_hf")
        nc.vector.tensor_copy(wo_hf, wo_f)
        bq_sb = const.tile([128, 2], F32, name="bq_sb")
        nc.sync.dma_start(bq_sb, bq_d)
        bk_sb = const.tile([128, 2], F32, name="bk_sb")
        nc.sync.dma_start(bk_sb, bk_d)
        bo_f = const.tile([1, HID], F32, name="bo_f")
        nc.scalar.dma_start(bo_f, bo_d)
        bo_hf = const.tile([1, HID], F16, name="bo_hf")
        nc.vector.tensor_copy(bo_hf, bo_f)

        identity = const.tile([128, 128], F32, name="identity")
        make_identity(nc, identity)
        ident_h = const.tile([128, 128], F16, name="ident_h")
        nc.vector.tensor_copy(ident_h, identity)
        ones_hf = const.tile([1, 128], F16, name="ones_hf")
        nc.vector.memset(ones_hf, 1.0)
        # K=2 broadcast selector at partitions 32/33 (value 1/2048 = 2^-11)
        sel2 = const.tile([128, 128], F16, name="sel2")
        nc.sync.dma_start(sel2[32:34, :], sel_d)

        # ---------------- persistent buffers ----------------
        qT_sb = [persist.tile([128, SQ], F16, name=f"qT_sb{g}") for g in range(2)]
        kT_ch = [[persist.tile([128, 512], F16, name=f"kT{g}_{c}")
                  for c in range(skc // 512)] for g in range(2)]
        # augmented V: per head 64 cols = [32 v | mask | 31 zeros]
        vaug = [persist.tile([128, 512], F16, name=f"vaug{s}")
                for s in range(nkc)]
        wtn_all = [persist.tile([128, 512], F16, name=f"wtn{i}")
                   for i in range(4)]
        # vaug init from host (mask col + zeros; v cols overwritten by proj)
        for s in range(nkc):
            nc.scalar.dma_start(vaug[s], vg_d[:, s * 512:(s + 1) * 512])

        # ---------------- phase A helpers ----------------
        def transpose_half(xins, col0, fast=False):
            """4 tiles -> fp16 [128, 512] chunk of rows col0..col0+128."""
            pool = wt_pool if fast else tpsum
            tp = pool.tile([128, 512], F16, tag="wt" if fast else "tp",
                           name="tp")
            for j in range(4):
                nc.tensor.transpose(
                    tp[:, j * 128:(j + 1) * 128],
                    xins[j][:, col0:col0 + 128], ident_h)
            ch = xt_pool.tile([128, 512], F16, tag="xch", name="xch")
            nc.vector.tensor_copy(ch, tp)
            return ch

        def project_chunk(chunks, w_hf, b_sb, outs, fast=False):
            for g in range(2):
                if fast:
                    # st pool is idle pre-attention; borrow it for depth-2
                    ps = st_pool.tile([128, 1024], F32, tag="st",
                                      name="ps")[:, 0:512]
                else:
                    ps = ppsum.tile([128, 512], F32, tag="proj", name="ps")
                for t in range(2):
                    nc.tensor.matmul(
                        ps, w_hf[t][:, g * 128:(g + 1) * 128], chunks[t],
                        start=(t == 0), stop=(t == 1))
                nc.scalar.activation(outs[g], ps, AF.Identity,
                                     bias=b_sb[:, g:g + 1], scale=1.0)

        def value_tile(s, xkv):
            vtp = tpsum.tile([128, 256], F16, tag="tp", name="vtp")
            for t in range(2):
                nc.tensor.transpose(
                    vtp[:, t * 128:(t + 1) * 128],
                    xkv[:, 256 + t * 128:256 + (t + 1) * 128], ident_h)
            vT = vt_pool.tile([128, 256], F16, tag="vT", name="vT")
            nc.scalar.activation(vT, vtp, AF.Copy)
            vps = ppsum.tile([128, 256], F32, tag="proj", name="vps")
            for t in range(2):
                nc.tensor.matmul(
                    vps, vT[:, t * 128:(t + 1) * 128], wv_hf[t],
                    start=(t == 0), stop=(t == 1))
            dst = vaug[s].rearrange("p (h e) -> p h e", e=64)[:, :, 0:DH]
            src = vps.rearrange("p (h e) -> p h e", e=DH)
            nc.vector.tensor_copy(dst, src)

        # ---------------- phase A emission ----------------
        def emit_kv_chunk(cch, fast=False):
            xins = [xkv_tiles[cch * 4 + j] for j in range(4)]
            chunks = [transpose_half(xins, t * 128, fast=fast)
                      for t in range(2)]
            project_chunk(chunks, wk_hf, bk_sb,
                          [kT_ch[g][cch] for g in range(2)], fast=fast)
            for j in range(4):
                value_tile(cch * 4 + j, xins[j])

        nch = skc // 512
        for sg in range(SQ // 512):
            xq = xq_all[sg * 4:sg * 4 + 4]
            chunks = [transpose_half(xq, t * 128, fast=True)
                      for t in range(2)]
            project_chunk(chunks, wq_hf, bq_sb,
                          [qT_sb[g][:, sg * 512:(sg + 1) * 512]
                           for g in range(2)], fast=True)
        emit_kv_chunk(0, fast=True)

        # ---------------- phase B: attention ----------------
        def make_tail(qc, wcops, dpack):
            def tail():
                # reciprocal: magic + 2 Newton iterations, output 2048/d fp16
                r0i = dn_pool.tile([8, 512], I32, tag="dp", name="r0i")
                nc.vector.tensor_scalar(r0i, dpack.bitcast(I32), -1, MAGIC,
                                        op0=ALU.mult, op1=ALU.add)
                r0 = r0i.bitcast(F32)
                t1 = dn_pool.tile([8, 512], F32, tag="dp", name="t1")
                nc.vector.tensor_tensor(t1, dpack, r0, op=ALU.mult)
                t1b = dn_pool.tile([8, 512], F32, tag="dp", name="t1b")
                nc.vector.tensor_scalar(t1b, t1, -RS, 2.0 * RS,
                                        op0=ALU.mult, op1=ALU.add)
                r2h = dn_pool.tile([8, 512], F16, tag="dp", name="r2h")
                nc.vector.tensor_tensor(r2h, r0, t1b, op=ALU.mult)

                # broadcast + normalize
                for g in range(2):
                    for jj in range(2):
                        p = 2 * g + jj
                        rsp = rs_pool.tile([34, 512], F16, tag="rsp",
                                           name="rsp")
                        r = 4 * g + 2 * jj
                        nc.sync.dma_start(rsp[32:33, :], r2h[r:r + 1, :])
                        nc.sync.dma_start(rsp[33:34, :], r2h[r + 1:r + 2, :])
                        if qc == SQ // 512 - 1:
                            # final tail: st pool is free; depth-2 broadcast
                            bc = st_pool.tile([128, 1024], F32, tag="st",
                                              name="bc")[:, 0:512]
                        else:
                            bc = tpsum.tile([128, 512], F32, tag="tp",
                                            name="bc")
                        nc.tensor.matmul(bc, sel2[32:34, :], rsp[32:34, :],
                                         start=True, stop=True,
                                         tile_position=(32, 0))
                        nc.vector.tensor_tensor(wtn_all[p], wcops[p], bc,
                                                op=ALU.mult)

                # output projection (final tail: st pool is free, depth 2)
                for m in range(4):
                    if qc == SQ // 512 - 1:
                        ops = st_pool.tile([128, 1024], F32, tag="st",
                                           name="ops")[:, 0:256]
                    else:
                        ops = ppsum.tile([128, 256], F32, tag="proj",
                                         name="ops")
                    for p in range(4):
                        nc.tensor.matmul(
                            ops, wtn_all[p][:, m * 128:(m + 1) * 128],
                            wo_hf[:, p * 256:(p + 1) * 256],
                            start=(p == 0), stop=False,
                            skip_group_check=True)
                    nc.tensor.matmul(ops, ones_hf[0:1, :], bo_hf,
                                     start=False, stop=True,
                                     skip_group_check=True)
                    ob = osb_pool.tile([128, 256], F32, tag="ob", name="ob")
                    nc.scalar.activation(ob, ops, AF.Copy)
                    nc.sync.dma_start(
                        out_d[qc * 512 + m * 128:qc * 512 + (m + 1) * 128, :],
                        ob)
            return tail

        pending_tail = None
        for qc in range(SQ // 512):
            wcops = []
            dpack = dn_pool.tile([8, 512], F32, tag="dp", name="dpack")
            for g in range(2):
                wts = [wt_pool.tile([128, 512], F32, tag="wt", name=f"wt{jj}")
                       for jj in range(2)]
                for kt in range(nkc):
                    # all four score matmuls back-to-back: disjoint 32-row
                    # strips run concurrently on the PE (one wave, not two)
                    sts = [st_pool.tile([128, 1024], F32, tag="st",
                                        name="st") for _ in range(2)]
                    for jj in range(2):
                        for j2 in range(2):
                            j = 2 * jj + j2
                            nc.tensor.matmul(
                                sts[jj][:, j2 * 512:(j2 + 1) * 512],
                                kT_ch[g][kt // 4][32 * j:32 * j + 32,
                                                  (kt % 4) * 128:
                                                  (kt % 4) * 128 + 128],
                                qT_sb[g][32 * j:32 * j + 32,
                                         qc * 512:(qc + 1) * 512],
                                start=True, stop=True,
                                tile_position=(32 * j, 0))
                    for jj in range(2):
                        st = sts[jj]
                        if _use_dve(qc, g, jj):
                            p1 = pt_pool.tile([128, 1024], I16, tag="pt",
                                              name="p1")
                            nc.vector.tensor_scalar(
                                p1, st, ASC, B16,
                                op0=ALU.mult, op1=ALU.add)
                            p2 = pt_pool.tile([128, 1024], I16, tag="pt",
                                              name="p2")
                            nc.gpsimd.tensor_scalar(
                                p2, p1, 1, 512,
                                op0=ALU.mult, op1=ALU.add)
                            streams = [p1.bitcast(F16), p2.bitcast(F16)]
                        else:
                            pt = pt_pool.tile([128, 1024], F16, tag="pt",
                                              name="pt")
                            nc.scalar.activation(pt, st, AF.Exp,
                                                 scale=SCALE)
                            streams = [pt]
                        nstr = len(streams)
                        for si, pstr in enumerate(streams):
                            for j2 in range(2):
                                h = 4 * g + 2 * jj + j2
                                nc.tensor.matmul(
                                    wts[jj][64 * j2:64 * j2 + 64, :],
                                    vaug[kt][:, 64 * h:64 * h + 64],
                                    pstr[:, j2 * 512:(j2 + 1) * 512],
                                    start=(kt == 0 and si == 0),
                                    stop=(kt == nkc - 1 and si == nstr - 1),
                                    tile_position=(0, 64 * j2),
                                    skip_group_check=True)

                    # interleave remaining phase-A work into the first group
                    if qc == 0 and g == 0:
                        c = kt // 4
                        if kt % 4 == 3 and (c + 2) < nch:
                            for s in range((c + 2) * 4, (c + 3) * 4):
                                if s not in xkv_tiles:
                                    gather_kv(s)
                        if kt % 4 == 2 and (c + 1) < nch:
                            emit_kv_chunk(c + 1)


                # evict (frees PSUM), stash denominator rows
                for jj in range(2):
                    wcop = wc_pool.tile([128, 512], F32, tag="wcop",
                                        name="wcop")
                    nc.scalar.activation(wcop, wts[jj], AF.Copy)
                    r = 4 * g + 2 * jj
                    nc.sync.dma_start(dpack[r:r + 1, :], wcop[32:33, :])
                    nc.sync.dma_start(dpack[r + 1:r + 2, :], wcop[96:97, :])
                    wcops.append(wcop)

                # overlap previous qc's normalize+outproj under this qc
                if pending_tail is not None and g == 0:
                    pending_tail()
                    pending_tail = None

            pending_tail = make_tail(qc, wcops, dpack)
        pending_tail()

    nc.finalize()
    return nc


def _get_nc(nkc):
    key = ("nc", nkc)
    if key not in _CACHE:
        _CACHE[key] = _build_nc(nkc)
    return _CACHE[key]


def kernel(query, key, value, mask, Wq, bq, Wk, bk, Wv, bv, Wo, bo,
           _trace=False):
    query = np.asarray(query, np.float32)
    key = np.asarray(key, np.float32)
    value = np.asarray(value, np.float32)
    mask = np.asarray(mask, np.int32)
    Wq = np.ascontiguousarray(np.asarray(Wq, np.float32))
    Wk = np.ascontiguousarray(np.asarray(Wk, np.float32))
    Wv = np.ascontiguousarray(np.asarray(Wv, np.float32))
    Wo = np.ascontiguousarray(np.asarray(Wo, np.float32))
    bq = np.asarray(bq, np.float32)
    bk = np.asarray(bk, np.float32)
    bv = np.asarray(bv, np.float32)
    bo = np.asarray(bo, np.float32)

    # mask compaction: indices of surviving keys per batch, padded to a
    # multiple of 512 with pointers at the zero rows
    idxs = [np.nonzero(mask[b, 0])[0].astype(np.int32) for b in range(2)]
    nk_max = max(len(ix) for ix in idxs)
    nk_max = max(nk_max, 1)
    skc = ((nk_max + 511) // 512) * 512
    nkc = skc // 128

    nc = _get_nc(nkc)

    wo_arr = np.zeros((128, 4, 256), np.float32)
    for p in range(4):
        wo_arr[0:32, p] = Wo[64 * p:64 * p + 32]
        wo_arr[64:96, p] = Wo[64 * p + 32:64 * p + 64]
    wo_arr = np.ascontiguousarray(wo_arr.reshape(128, 1024))
    bq2 = np.ascontiguousarray(bq.reshape(2, 128).T)
    bk2 = np.ascontiguousarray(bk.reshape(2, 128).T)
    bo2 = np.ascontiguousarray((bv @ Wo + bo).reshape(1, 256))
    sel2 = np.zeros((2, 128), np.float16)
    sel2[0, 0:32] = 1.0 / RS
    sel2[1, 64:96] = 1.0 / RS

    # per-batch KV concat with zero pad rows (fp16 on host)
    kv_full = []
    for b in range(2):
        kv = np.zeros((SKP, 2 * HID), np.float16)
        kv[:SK, :HID] = key[b].astype(np.float16)
        kv[:SK, HID:] = value[b].astype(np.float16)
        kv_full.append(kv)

    in_maps = []
    for c in range(8):
        b, qi = divmod(c, 4)
        ix = idxs[b]
        nk = len(ix)
        ix_pad = np.concatenate(
            [ix, np.full(skc - nk, SK, np.int32)])
        mcomp = (np.arange(skc) < nk).astype(np.float16)
        ib = np.ascontiguousarray(ix_pad.reshape(nkc, 128).T)
        # vaug init pattern: per tile, per head: [32 zeros | mask | 31 zeros]
        mb = np.ascontiguousarray(mcomp.reshape(nkc, 128).T)  # [128, nkc]
        vgi = np.zeros((128, nkc, 8, 64), np.float16)
        vgi[:, :, :, 32] = mb[:, :, None]
        vgi = np.ascontiguousarray(vgi.reshape(128, nkc * 512))
        in_maps.append({
            "q_in": np.ascontiguousarray(
                query[b, qi * SQ:(qi + 1) * SQ].astype(np.float16)),
            "kv_in": kv_full[b],
            "vginit": vgi,
            "idx_in": ib,
            "wq": Wq, "wk": Wk, "wv": Wv, "wo_arr": wo_arr,
            "bq2": bq2, "bk2": bk2, "bo2": bo2, "sel2": sel2,
        })

    res = run_bass_kernel_spmd(nc, in_maps, core_ids=list(range(8)),
                               trace=_trace)
    if _trace:
        _CACHE["last_result"] = res

    out = np.empty((2, 4096, HID), np.float32)
    for c in range(8):
        b, qi = divmod(c, 4)
        out[b, qi * SQ:(qi + 1) * SQ] = res.results[c]["out"]
    return out



# revision 10
# speedup vs baseline: 1.3329x; 1.3329x over previous
"""Trainium2 Bass kernel for nn_AttentionBlock (B=2, S=4096, HID=256, 8 heads).

Sharding: 8 cores = 2 batches x 4 query-chunks of 1024 queries.
Each core redundantly computes K/V projections for its batch over the
mask-compacted key set, then attention for its 1024 queries over all 8
heads, then the output projection. Host gathers by concatenation.

v4 structure (host-layout + lean exp pipeline + software-pipelined PE):
- Host pre-compacts keys (mask nonzero indices), pre-transposes q/k/v to
  hid-major fp16, pre-casts weights to fp16. No device-side gathers or
  transposes; phase A is pure projection matmuls.
- Key-tile count nkc = ceil(nk_max/128) (128-granular, not 512).
- qT is pre-scaled by ASC = (1024/ln2)/sqrt(32) at projection eviction, so
  score PSUM holds t = ASC*x. Exp is one op per [128,1024] tile,
  alternating engines per key-tile for balance:
    ACT: LUT exp with scale=ln2/1024 (exact)
    DVE: tensor_scalar +B16C -> int16, bitcast fp16 (single-sample
         Schraudolph, geometrically centered, +-2.98% sawtooth that
         averages out over ~2k keys)
- PV matmuls vs mask-augmented V tiles ([32 v | mask | 31 zeros] per head)
  accumulate numerators and denominators together: 2 col-packed waves per
  key-tile; 3 PE waves per (g,kt) cycle total.
- PE stream is software-pipelined: scores(kt+1) issue before PV(kt) so the
  PE never head-of-line blocks on the exp engines.
- PSUM: 3x [128,1024]f32 score tiles (6 banks) + 2 wt accumulators.
  Phase A projections and tail broadcast/outproj borrow score-pool tiles.
- Tail per 512-query chunk: denominator rows DMA-packed, reciprocal via
  int32 magic + 1 Newton step, broadcast to 128 partitions with one K=2
  matmul, normalize multiply on DVE, fused output projection against
  zero-padded Wo rows with bias via K=1 ones matmul. The qc0 tail's
  matmuls are deferred into qc1's pipeline; the final tail runs its g0
  half early (during g1 compute) to shorten the serial epilogue.
"""

import numpy as np

import concourse.bacc as bacc
import concourse.bass as bass
from concourse import mybir
from concourse.tile import TileContext
from concourse.bass_utils import run_bass_kernel_spmd

F32 = mybir.dt.float32
F16 = mybir.dt.float16
I16 = mybir.dt.int16
I32 = mybir.dt.int32
AF = mybir.ActivationFunctionType
ALU = mybir.AluOpType

HID = 256
HEADS = 8
DH = 32
SK = 4096
SQ = 1024   # queries per core
SCALE = 1.0 / np.sqrt(32.0)
A16 = 1024.0 / np.log(2.0)          # Schraudolph slope (fp16 format)
ASC = float(A16 * SCALE)            # folded into qT at projection
EXPS = float(np.log(2.0) / 1024.0)  # ACT exp scale on t-space scores
B16 = 15360.0                       # Schraudolph offset (15*1024)
# single-sample Schraudolph: center the sawtooth geometrically so the
# per-key multiplicative error is zero-mean (range +-2.98%)
B16C = float(B16 - 1024.0 * np.log2(1.0614) / 2.0)
MAGIC = 0x7EF311C3                  # fp32 reciprocal magic
RS = 2048.0                         # reciprocal output scaling

_CACHE = {}


def _build_nc(nkc):
    """nkc = number of 128-key tiles after mask compaction."""
    nch = (nkc + 3) // 4           # 512-key projection chunks
    skp = nch * 512                # padded key columns in kT/vT inputs
    nc = bacc.Bacc("TRN2", target_bir_lowering=False, debug=False,
                   num_devices=8)

    q_d = nc.dram_tensor("qt_in", [HID, SQ], F16, kind="ExternalInput").ap()
    k_d = nc.dram_tensor("kt_in", [HID, skp], F16, kind="ExternalInput").ap()
    v_d = nc.dram_tensor("vt_in", [HID, skp], F16, kind="ExternalInput").ap()
    wq_d = nc.dram_tensor("wq", [HID, HID], F16, kind="ExternalInput").ap()
    wk_d = nc.dram_tensor("wk", [HID, HID], F16, kind="ExternalInput").ap()
    wv_d = nc.dram_tensor("wv", [HID, HID], F16, kind="ExternalInput").ap()
    wo_d = nc.dram_tensor("wo_arr", [128, 1024], F16, kind="ExternalInput").ap()
    bq_d = nc.dram_tensor("bq2", [128, 2], F32, kind="ExternalInput").ap()
    bk_d = nc.dram_tensor("bk2", [128, 2], F32, kind="ExternalInput").ap()
    bo_d = nc.dram_tensor("bo2", [1, HID], F16, kind="ExternalInput").ap()
    sel_d = nc.dram_tensor("sel2", [2, 128], F16, kind="ExternalInput").ap()
    vm_d = nc.dram_tensor("vm8", [128, nkc * 8], F16,
                          kind="ExternalInput").ap()
    out_d = nc.dram_tensor("out", [SQ, HID], F32, kind="ExternalOutput").ap()

    from contextlib import ExitStack

    with TileContext(nc) as tc, ExitStack() as top:
        const = top.enter_context(tc.tile_pool(name="const", bufs=1))
        persist = top.enter_context(tc.tile_pool(name="persist", bufs=1))
        pt_pool = top.enter_context(tc.tile_pool(name="pt", bufs=4))
        wc_pool = top.enter_context(tc.tile_pool(name="wc", bufs=8))
        dn_pool = top.enter_context(tc.tile_pool(name="dn", bufs=16))
        rs_pool = top.enter_context(tc.tile_pool(name="rs", bufs=8))
        osb_pool = top.enter_context(tc.tile_pool(name="osb", bufs=4))

        st_pool = top.enter_context(tc.tile_pool(name="stp", bufs=3,
                                                 space="PSUM"))
        wt_pool = top.enter_context(tc.tile_pool(name="wtp", bufs=2,
                                                 space="PSUM"))

        # round-robin DMA issue across engines (all idle at startup)
        dma_engines = [nc.sync, nc.scalar, nc.gpsimd]
        dma_i = [0]

        def dma(dst, src):
            e = dma_engines[dma_i[0] % len(dma_engines)]
            dma_i[0] += 1
            e.dma_start(dst, src)

        # ------------- inputs in consumption order -------------
        wq_sb = []
        wk_sb = []
        wv_sb = []
        for t in range(2):
            for nm, d_ap, lst in (("wk", wk_d, wk_sb), ("wv", wv_d, wv_sb),
                                  ("wq", wq_d, wq_sb)):
                wb = const.tile([128, 256], F16, name=f"{nm}_h{t}")
                dma(wb, d_ap[t * 128:(t + 1) * 128, :])
                lst.append(wb)
        bq_sb = const.tile([128, 2], F32, name="bq_sb")
        dma(bq_sb, bq_d)
        bk_sb = const.tile([128, 2], F32, name="bk_sb")
        dma(bk_sb, bk_d)

        k_raw = [[None] * nch for _ in range(2)]
        v_raw = [[None] * nch for _ in range(2)]

        def load_chunk(c):
            for t in range(2):
                xk = const.tile([128, 512], F16, name=f"kraw{t}_{c}")
                dma(xk, k_d[t * 128:(t + 1) * 128, c * 512:(c + 1) * 512])
                k_raw[t][c] = xk
                xv = const.tile([128, 512], F16, name=f"vraw{t}_{c}")
                dma(xv, v_d[t * 128:(t + 1) * 128, c * 512:(c + 1) * 512])
                v_raw[t][c] = xv

        load_chunk(0)
        q_raw = []
        for t in range(2):
            xq = const.tile([128, SQ], F16, name=f"qraw{t}")
            dma(xq, q_d[t * 128:(t + 1) * 128, :])
            q_raw.append(xq)
        vm_sb = const.tile([128, nkc * 8], F16, name="vm_sb")
        dma(vm_sb, vm_d)
        wo_sb = const.tile([128, 1024], F16, name="wo_sb")
        dma(wo_sb, wo_d)
        bo_sb = const.tile([1, HID], F16, name="bo_sb")
        dma(bo_sb, bo_d)
        sel2 = const.tile([128, 128], F16, name="sel2")
        dma(sel2[32:34, :], sel_d)
        for c in range(1, nch):
            load_chunk(c)

        ones_hf = const.tile([1, 128], F16, name="ones_hf")
        nc.vector.memset(ones_hf, 1.0)
        # preload the exp activation table while DMAs are in flight
        dumm = const.tile([1, 8], F32, name="dumm")
        nc.vector.memset(dumm, 0.0)
        dummo = const.tile([1, 8], F16, name="dummo")
        nc.scalar.activation(dummo, dumm, AF.Exp, scale=1.0)

        # ---------------- persistent buffers ----------------
        qT_sb = [persist.tile([128, SQ], F16, name=f"qT_sb{g}")
                 for g in range(2)]
        kT_ch = [[persist.tile([128, 512], F16, name=f"kT{g}_{c}")
                  for c in range(nch)] for g in range(2)]
        # augmented V: per head 64 cols = [32 v | mask | 31 zeros]
        vaug_all = persist.tile([128, nkc * 512], F16, name="vaug")
        vaug = [vaug_all[:, s * 512:(s + 1) * 512] for s in range(nkc)]
        wtn_all = [persist.tile([128, 512], F16, name=f"wtn{i}")
                   for i in range(4)]
        nc.vector.memset(vaug_all, 0.0)
        vdst = vaug_all.rearrange("p (s h e) -> p s h e", h=8, e=64)
        nc.vector.tensor_copy(
            vdst[:, :, :, 32:33],
            vm_sb.rearrange("p (s h e) -> p s h e", h=8, e=1))

        # ---------------- phase A helpers ----------------
        def emit_k_chunk(c):
            for g in range(2):
                ps = st_pool.tile([128, 1024], F32, tag="st",
                                  name="kps")[:, 0:512]
                for t in range(2):
                    nc.tensor.matmul(
                        ps, wk_sb[t][:, g * 128:(g + 1) * 128], k_raw[t][c],
                        start=(t == 0), stop=(t == 1))
                nc.scalar.activation(kT_ch[g][c], ps, AF.Identity,
                                     bias=bk_sb[:, g:g + 1], scale=1.0)

        def emit_v_tile(s):
            c, r = divmod(s, 4)
            ps = st_pool.tile([128, 1024], F32, tag="st",
                              name="vps")[:, 0:256]
            for t in range(2):
                nc.tensor.matmul(
                    ps, v_raw[t][c][:, r * 128:(r + 1) * 128], wv_sb[t],
                    start=(t == 0), stop=(t == 1))
            dst = vaug[s].rearrange("p (h e) -> p h e", e=64)[:, :, 0:DH]
            src = ps.rearrange("p (h e) -> p h e", e=DH)
            nc.vector.tensor_copy(dst, src)

        def emit_q():
            for g in range(2):
                for cq in range(2):
                    ps = st_pool.tile([128, 1024], F32, tag="st",
                                      name="qps")[:, 0:512]
                    for t in range(2):
                        nc.tensor.matmul(
                            ps, wq_sb[t][:, g * 128:(g + 1) * 128],
                            q_raw[t][:, cq * 512:(cq + 1) * 512],
                            start=(t == 0), stop=(t == 1))
                    nc.scalar.activation(
                        qT_sb[g][:, cq * 512:(cq + 1) * 512], ps,
                        AF.Identity, bias=bq_sb[:, g:g + 1], scale=ASC)

        # chunk 0 + queries up front; chunks 1.. interleave into qc0/g0
        emit_k_chunk(0)
        for s in range(min(4, nkc)):
            emit_v_tile(s)
        emit_q()

        # ---------------- attention building blocks ----------------
        def emit_scores(qc, g, kt):
            c, r = divmod(kt, 4)
            sts = [st_pool.tile([128, 1024], F32, tag="st", name="st")
                   for _ in range(2)]
            for jj in range(2):
                for j2 in range(2):
                    j = 2 * jj + j2
                    nc.tensor.matmul(
                        sts[jj][:, j2 * 512:(j2 + 1) * 512],
                        kT_ch[g][c][32 * j:32 * j + 32,
                                    r * 128:r * 128 + 128],
                        qT_sb[g][32 * j:32 * j + 32,
                                 qc * 512:(qc + 1) * 512],
                        start=True, stop=True,
                        tile_position=(32 * j, 0))
            return sts

        def emit_exp(kt, sts):
            pts = []
            for jj in range(2):
                pt = pt_pool.tile([128, 1024], F16, tag="pt", name="pt")
                if (kt + jj) % 2 == 0:
                    nc.vector.tensor_scalar(
                        pt.bitcast(I16), sts[jj], 1.0, B16C,
                        op0=ALU.mult, op1=ALU.add)
                else:
                    nc.scalar.activation(pt, sts[jj], AF.Exp, scale=EXPS)
                pts.append(pt)
            return pts

        def emit_pv(g, kt, wts, pts):
            for jj in range(2):
                for j2 in range(2):
                    h = 4 * g + 2 * jj + j2
                    nc.tensor.matmul(
                        wts[jj][64 * j2:64 * j2 + 64, :],
                        vaug[kt][:, 64 * h:64 * h + 64],
                        pts[jj][:, j2 * 512:(j2 + 1) * 512],
                        start=(kt == 0), stop=(kt == nkc - 1),
                        tile_position=(0, 64 * j2),
                        skip_group_check=True)

        def emit_wcop(g, wts, wcops, dpack):
            for jj in range(2):
                wcop = wc_pool.tile([128, 512], F32, tag="wcop",
                                    name="wcop")
                nc.scalar.activation(wcop, wts[jj], AF.Copy)
                r = 2 * jj
                nc.sync.dma_start(dpack[r:r + 1, :], wcop[32:33, :])
                nc.sync.dma_start(dpack[r + 1:r + 2, :], wcop[96:97, :])
                wcops.append(wcop)

        def emit_recip(eng, dpack):
            """reciprocal of dpack [4,512] -> 2048/d fp16; 4 serial ops."""
            r0i = dn_pool.tile([4, 512], I32, tag="dp", name="r0i")
            eng.tensor_scalar(r0i, dpack.bitcast(I32), -1, MAGIC,
                              op0=ALU.mult, op1=ALU.add)
            r0 = r0i.bitcast(F32)
            t1 = dn_pool.tile([4, 512], F32, tag="dp", name="t1")
            eng.tensor_tensor(t1, dpack, r0, op=ALU.mult)
            t1b = dn_pool.tile([4, 512], F32, tag="dp", name="t1b")
            eng.tensor_scalar(t1b, t1, -RS, 2.0 * RS,
                              op0=ALU.mult, op1=ALU.add)
            r2h = dn_pool.tile([4, 512], F16, tag="dp", name="r2h")
            eng.tensor_tensor(r2h, r0, t1b, op=ALU.mult)
            return r2h

        def emit_norm(p, r2h, ra, wcop):
            """broadcast reciprocal rows + normalize weight copy p."""
            rsp = rs_pool.tile([34, 512], F16, tag="rsp", name="rsp")
            nc.sync.dma_start(rsp[32:33, :], r2h[ra:ra + 1, :])
            nc.sync.dma_start(rsp[33:34, :], r2h[ra + 1:ra + 2, :])
            bc = st_pool.tile([128, 1024], F32, tag="st", name="bc")[:, 0:512]
            nc.tensor.matmul(bc, sel2[32:34, :], rsp[32:34, :],
                             start=True, stop=True, tile_position=(32, 0))
            nc.vector.tensor_tensor(wtn_all[p], wcop, bc, op=ALU.mult)

        def emit_outproj(qc):
            for m in range(4):
                ops = st_pool.tile([128, 1024], F32, tag="st",
                                   name="ops")[:, 0:256]
                for p in range(4):
                    nc.tensor.matmul(
                        ops, wtn_all[p][:, m * 128:(m + 1) * 128],
                        wo_sb[:, p * 256:(p + 1) * 256],
                        start=(p == 0), stop=False, skip_group_check=True)
                nc.tensor.matmul(ops, ones_hf[0:1, :], bo_sb,
                                 start=False, stop=True,
                                 skip_group_check=True)
                ob = osb_pool.tile([128, 256], F32, tag="ob", name="ob")
                nc.scalar.activation(ob, ops, AF.Copy)
                nc.sync.dma_start(
                    out_d[qc * 512 + m * 128:qc * 512 + (m + 1) * 128, :],
                    ob)

        # ---------------- attention main loop ----------------
        tail_mm = None      # deferred matmul part of the qc0 tail
        prev_qc = {}
        for qc in range(SQ // 512):
            wcops = []
            dpacks = [dn_pool.tile([4, 512], F32, tag="dp", name="dpack")
                      for _ in range(2)]
            for g in range(2):
                wts = [wt_pool.tile([128, 512], F32, tag="wt",
                                    name=f"wt{jj}") for jj in range(2)]
                prev = None
                for kt in range(nkc):
                    sts = emit_scores(qc, g, kt)
                    pts = emit_exp(kt, sts)
                    if prev is not None:
                        emit_pv(g, kt - 1, wts, prev)
                    prev = pts

                    # interleave remaining phase-A work into qc0/g0
                    if qc == 0 and g == 0 and kt % 4 == 2:
                        cc = kt // 4 + 1
                        if cc < nch:
                            emit_k_chunk(cc)
                            for s2 in range(cc * 4, min((cc + 1) * 4, nkc)):
                                emit_v_tile(s2)
                    # deferred tail matmuls of the previous qc
                    if tail_mm is not None and g == 0 and kt == 2:
                        tail_mm()
                        tail_mm = None
                emit_pv(g, nkc - 1, wts, prev)
                emit_wcop(g, wts, wcops, dpacks[g])

                if qc == SQ // 512 - 1 and g == 0:
                    # final tail: run the g0 half early, during g1 compute
                    r2h_a = emit_recip(nc.vector, dpacks[0])
                    for jj in range(2):
                        emit_norm(jj, r2h_a, 2 * jj, wcops[jj])
            prev_qc[qc] = (wcops, dpacks)

            if qc < SQ // 512 - 1:
                def make_tail(qc0, wcops0, dpacks0):
                    r2h_g = [emit_recip(nc.gpsimd, dpacks0[g2])
                             for g2 in range(2)]

                    def mm_part():
                        for g2 in range(2):
                            for jj in range(2):
                                p = 2 * g2 + jj
                                emit_norm(p, r2h_g[g2], 2 * jj, wcops0[p])
                        emit_outproj(qc0)
                    return mm_part
                tail_mm = make_tail(qc, wcops, dpacks)

        # final epilogue: g1 half of the last tail + output projection
        wcops, dpacks = prev_qc[SQ // 512 - 1]
        r2h_b = emit_recip(nc.vector, dpacks[1])
        for jj in range(2):
            emit_norm(2 + jj, r2h_b, 2 * jj, wcops[2 + jj])
        emit_outproj(SQ // 512 - 1)

    nc.finalize()
    return nc


def _get_nc(nkc):
    key = ("nc", nkc)
    if key not in _CACHE:
        _CACHE[key] = _build_nc(nkc)
    return _CACHE[key]


def kernel(query, key, value, mask, Wq, bq, Wk, bk, Wv, bv, Wo, bo,
           _trace=False):
    query = np.asarray(query, np.float32)
    key = np.asarray(key, np.float32)
    value = np.asarray(value, np.float32)
    mask = np.asarray(mask, np.int32)
    Wq = np.asarray(Wq, np.float32)
    Wk = np.asarray(Wk, np.float32)
    Wv = np.asarray(Wv, np.float32)
    Wo = np.asarray(Wo, np.float32)
    bq = np.asarray(bq, np.float32)
    bk = np.asarray(bk, np.float32)
    bv = np.asarray(bv, np.float32)
    bo = np.asarray(bo, np.float32)

    # mask compaction: indices of surviving keys per batch
    idxs = [np.nonzero(mask[b, 0])[0].astype(np.int32) for b in range(2)]
    nk_max = max(max(len(ix) for ix in idxs), 1)
    nkc = max((nk_max + 127) // 128, 4)
    nch = (nkc + 3) // 4
    skp = nch * 512

    nc = _get_nc(nkc)

    wo_arr = np.zeros((128, 4, 256), np.float32)
    for p in range(4):
        wo_arr[0:32, p] = Wo[64 * p:64 * p + 32]
        wo_arr[64:96, p] = Wo[64 * p + 32:64 * p + 64]
    wo_arr = np.ascontiguousarray(
        wo_arr.reshape(128, 1024).astype(np.float16))
    bq2 = np.ascontiguousarray(bq.reshape(2, 128).T * ASC).astype(np.float32)
    bk2 = np.ascontiguousarray(bk.reshape(2, 128).T).astype(np.float32)
    bo2 = np.ascontiguousarray(
        (bv @ Wo + bo).reshape(1, 256)).astype(np.float16)
    sel2 = np.zeros((2, 128), np.float16)
    sel2[0, 0:32] = 1.0 / RS
    sel2[1, 64:96] = 1.0 / RS
    wq16 = np.ascontiguousarray(Wq.astype(np.float16))
    wk16 = np.ascontiguousarray(Wk.astype(np.float16))
    wv16 = np.ascontiguousarray(Wv.astype(np.float16))

    # per-batch compacted, hid-major k/v + query transposes
    kT_b = []
    vT_b = []
    vm_b = []
    for b in range(2):
        ix = idxs[b]
        nk = len(ix)
        kc = np.zeros((skp, HID), np.float16)
        kc[:nk] = key[b][ix].astype(np.float16)
        vc = np.zeros((skp, HID), np.float16)
        vc[:nk] = value[b][ix].astype(np.float16)
        kT_b.append(np.ascontiguousarray(kc.T))
        vT_b.append(np.ascontiguousarray(vc.T))
        mrow = (np.arange(nkc * 128) < nk).astype(np.float16)
        vm = np.repeat(mrow.reshape(nkc, 128, 1), 8, axis=2)  # [nkc,128,8]
        vm_b.append(np.ascontiguousarray(
            vm.transpose(1, 0, 2).reshape(128, nkc * 8)))

    in_maps = []
    for cidx in range(8):
        b, qi = divmod(cidx, 4)
        in_maps.append({
            "qt_in": np.ascontiguousarray(
                query[b, qi * SQ:(qi + 1) * SQ].astype(np.float16).T),
            "kt_in": kT_b[b],
            "vt_in": vT_b[b],
            "wq": wq16, "wk": wk16, "wv": wv16, "wo_arr": wo_arr,
            "bq2": bq2, "bk2": bk2, "bo2": bo2, "sel2": sel2,
            "vm8": vm_b[b],
        })

    res = run_bass_kernel_spmd(nc, in_maps, core_ids=list(range(8)),
                               trace=_trace)
    if _trace:
        _CACHE["last_result"] = res

    out = np.empty((2, 4096, HID), np.float32)
    for cidx in range(8):
        b, qi = divmod(cidx, 4)
        out[b, qi * SQ:(qi + 1) * SQ] = res.results[cidx]["out"]
    return out
